# revision 12
# baseline (speedup 1.0000x reference)
"""TRN2 Bass kernel: 100 sequential Linear layers (y = x @ W^T + b).

Restructured via linearity: the whole network is one affine map
y = x @ M + c with M = W1^T @ ... @ W100^T and c the propagated bias
chain. The chain is contractive (each W ~ U(-1/sqrt(D))), so this
association is numerically benign (verified ~2e-3 rel err in bf16).

Plan (single SPMD launch, 8 cores):
  phase 1  core i composes its ~13-layer segment into an affine
           (T_i = M_i^T stored [out,in] in bf16, c_i fp32), using
           512-wide bf16 matmuls for T and 1-wide matmuls for c.
  gather   PE-transpose T_i -> P_i = M_i ([in,out]); AllGather P (bf16)
           and c (fp32) through DRAM bounce buffers.
  combine  every core redundantly folds the 8 segment affines into
           (M_total, c_total): 7 compose steps of 16 matmuls each.
  apply    y_shard^T = M_total^T-free apply: 64 bf16 matmuls on the
           core's 2048-row batch shard + bias add fused into the
           PSUM->SBUF copy; results stream out as fp32.

Per-core PE work drops from ~3.3M cycles (data-parallel baseline) to
~210k cycles; weight DMA drops from 105MB fp32 to ~6.8MB bf16.
"""
import os
import sys
import types
import numpy as np
from ml_dtypes import bfloat16


def _ensure_ntff_hook():
    """Provide the antenv.axon_hooks registry this image lacks.

    trn_boot degrades silently when antenv.axon_hooks is missing, but
    bass_utils hard-imports it under trace=True. Recreate the tiny
    set/get registry and install the same ctypes-based hook trn_boot
    would have registered. No-op when tracing is unused.
    """
    try:
        import antenv.axon_hooks  # noqa: F401
        return
    except ImportError:
        pass
    try:
        import antenv
    except ImportError:
        return
    mod = types.ModuleType("antenv.axon_hooks")
    mod._hook = None

    def set_axon_ntff_profile_hook(h):
        mod._hook = h

    def get_axon_ntff_profile_hook():
        return mod._hook

    mod.set_axon_ntff_profile_hook = set_axon_ntff_profile_hook
    mod.get_axon_ntff_profile_hook = get_axon_ntff_profile_hook
    sys.modules["antenv.axon_hooks"] = mod
    antenv.axon_hooks = mod
    try:
        from trn_agent_boot.trn_boot import _ntff_profile_via_ctypes
        hook = _ntff_profile_via_ctypes("/opt/axon/libaxon_pjrt.so")
        if hook is not None:
            mod._hook = hook
    except Exception:
        pass


_ensure_ntff_hook()

import concourse.bacc as bacc
import concourse.mybir as mybir
import concourse.tile as tile
import concourse.bass_utils as bass_utils
from concourse.bass_utils import run_bass_kernel_spmd

f32 = mybir.dt.float32
bf16 = mybir.dt.bfloat16

N_CORES = 8
N_LAYERS = 100
D = 512
BATCH = 16384
B = BATCH // N_CORES   # 2048 rows per core
NT = D // 128          # 4 tiles of 128 over the hidden dim
NB = B // 512          # batch chunks of 512 (one PSUM bank each)
NCOMP = 12             # compose steps per core (identity-padded)
# segment layer counts: 4 cores of 13, 4 cores of 12 (= 100)
SEG_BOUNDS = [0, 13, 26, 39, 52, 64, 76, 88, 100]

LAST_EXEC_TIME_NS = None
LAST_RESULTS = None

# The axon trace path uploads profile artifacts to a fish bucket that is
# not reachable from this container; keep the artifacts local instead.
bass_utils.upload_artifacts = lambda d: d

_NC_CACHE = {}


def _build_nc():
    nc = bacc.Bacc("TRN2", target_bir_lowering=False, debug=False,
                   num_devices=N_CORES)
    # [out,in]-layout first layer of the segment (= Ws[l0] verbatim)
    T0 = nc.declare_dram_parameter("T0", [NT, 128, D], bf16, isOutput=False)
    # [in,out]-layout (d-major W^T) for layers l0+1.. (identity padded)
    WTs = nc.declare_dram_parameter("WTs", [NCOMP, NT, 128, D], bf16,
                                    isOutput=False)
    # biases: col m*4+j = b_seg[m][j*128:(j+1)*128]; m=0 is the init layer
    bsT = nc.declare_dram_parameter("bsT", [128, (NCOMP + 1) * NT], f32,
                                    isOutput=False)
    c0 = nc.declare_dram_parameter("c0", [128, NT], bf16, isOutput=False)
    xT = nc.declare_dram_parameter("xT", [NT, 128, B], bf16, isOutput=False)
    ident = nc.declare_dram_parameter("ident", [128, 128], bf16, isOutput=False)
    yT = nc.declare_dram_parameter("yT", [NT, 128, B], f32, isOutput=True)

    with tile.TileContext(nc) as tc:
        with tc.tile_pool(name="wpool", bufs=2) as w_pool, \
             tc.tile_pool(name="tpool", bufs=2) as t_pool, \
             tc.tile_pool(name="cpool", bufs=2) as c_pool, \
             tc.tile_pool(name="misc", bufs=1) as misc, \
             tc.tile_pool(name="ppool", bufs=2) as p_pool, \
             tc.tile_pool(name="ypool", bufs=2) as y_pool, \
             tc.tile_pool(name="psT", bufs=1, space="PSUM") as psT, \
             tc.tile_pool(name="psX", bufs=2, space="PSUM") as psX, \
             tc.tile_pool(name="psC", bufs=2, space="PSUM") as psC, \
             tc.tile_pool(name="dram", bufs=1, space="DRAM") as dram:

            # ---- input DMAs --------------------------------------------
            # weights stream on sync (even layers) + vector (odd) queues;
            # everything else on the scalar queue.
            Tcur = [t_pool.tile([128, D], bf16, name=f"T_in_{t}", tag=f"T{t}")
                    for t in range(NT)]
            for t in range(NT):
                nc.sync.dma_start(out=Tcur[t], in_=T0[t])

            Wtiles = []  # [m][t]
            for m in range(NCOMP):
                row = []
                for t in range(NT):
                    w = w_pool.tile([128, D], bf16, name=f"W_{m}_{t}",
                                    tag=f"W{t}")
                    q = nc.sync if (m % 2 == 0) else nc.scalar
                    q.dma_start(out=w, in_=WTs[m, t])
                    row.append(w)
                Wtiles.append(row)

            ident_sb = misc.tile([128, 128], bf16, name="ident_sb")
            nc.gpsimd.dma_start(out=ident_sb, in_=ident[:, :])
            bias_sb = misc.tile([128, (NCOMP + 1) * NT], f32, name="bias_sb")
            nc.gpsimd.dma_start(out=bias_sb, in_=bsT[:, :])
            c_cur = c_pool.tile([128, NT], bf16, name="c_in", tag="c")
            nc.gpsimd.dma_start(out=c_cur, in_=c0[:, :])
            Xt = [misc.tile([128, B], bf16, name=f"X_{t}") for t in range(NT)]
            for t in range(NT):
                nc.gpsimd.dma_start(out=Xt[t], in_=xT[t])

            # DRAM bounce buffers for the collectives
            p_in = dram.tile([NT * 128, D], bf16, name="p_in")
            p_out = dram.tile([N_CORES * NT * 128, D], bf16, name="p_out",
                              addr_space="Shared")
            c_in = dram.tile([128, NT], f32, name="c_in_d")
            c_out = dram.tile([N_CORES * 128, NT], f32, name="c_out_d",
                              addr_space="Shared")

            def compose_step(stat, Told, c_old, bias_ap, c_add, last_c_f32,
                             tagsuf):
                """T_new = stat^T-compose(Told); c_new = stat^T c_old (+bias).

                stat: list of NT tiles [128, D] ([in k, out j] layout)
                Told: list of NT moving tiles [128, D]
                bias_ap: [128, NT] fp32 AP added to the c psum (or None)
                c_add: extra [128, NT] fp32 AP (combine's c_s) or None
                """
                Tnew = [t_pool.tile([128, D], bf16, name=f"T{tagsuf}_{t}",
                                    tag=f"T{t}") for t in range(NT)]
                ps_c = psC.tile([128, NT], f32, name=f"psc{tagsuf}", tag="psc")
                for j in range(NT):
                    ps = psT.tile([128, D], f32, name=f"ps{tagsuf}_{j}",
                                  tag=f"psT{j}")
                    for k in range(NT):
                        nc.tensor.matmul(
                            ps, stat[k][:, j * 128:(j + 1) * 128], Told[k],
                            start=(k == 0), stop=(k == NT - 1))
                    # 1-wide c matmuls ride between the wide groups
                    for k in range(NT):
                        nc.tensor.matmul(
                            ps_c[:, j:j + 1],
                            stat[k][:, j * 128:(j + 1) * 128],
                            c_old[:, k:k + 1],
                            start=(k == 0), stop=(k == NT - 1))
                    eng = nc.vector if j % 2 == 0 else nc.scalar
                    if eng is nc.vector:
                        eng.tensor_copy(Tnew[j], ps)
                    else:
                        eng.copy(out=Tnew[j], in_=ps)
                c_new = c_pool.tile([128, NT], f32 if last_c_f32 else bf16,
                                    name=f"c{tagsuf}",
                                    tag="cf" if last_c_f32 else "c")
                add_ap = bias_ap if bias_ap is not None else c_add
                nc.vector.tensor_add(c_new, ps_c, add_ap)
                return Tnew, c_new

            # ---- phase 1: compose own segment --------------------------
            for m in range(NCOMP):
                Tcur, c_cur = compose_step(
                    Wtiles[m], Tcur, c_cur,
                    bias_sb[:, (m + 1) * NT:(m + 2) * NT], None,
                    last_c_f32=(m == NCOMP - 1), tagsuf=f"p1_{m}")
            c_i = c_cur  # fp32 [128, NT]

            # ship c_i out + gather (c first: it's ready before the transposes)
            nc.gpsimd.dma_start(out=c_in, in_=c_i)
            nc.gpsimd.collective_compute(
                "AllGather", mybir.AluOpType.bypass,
                replica_groups=[list(range(N_CORES))],
                ins=[c_in.opt()], outs=[c_out.opt()])

            # ---- transpose T_i -> P_i, ship + gather -------------------
            for r in range(NT):
                ps = psX.tile([128, D], bf16, name=f"psP_{r}", tag="psx")
                for cb in range(NT):
                    nc.tensor.transpose(
                        ps[:, cb * 128:(cb + 1) * 128],
                        Tcur[cb][:, r * 128:(r + 1) * 128], ident_sb)
                p_sb = p_pool.tile([128, D], bf16, name=f"P_{r}", tag=f"P{r}")
                eng = nc.vector if r % 2 == 0 else nc.scalar
                if eng is nc.vector:
                    eng.tensor_copy(p_sb, ps)
                else:
                    eng.copy(out=p_sb, in_=ps)
                nc.gpsimd.dma_start(out=p_in[r * 128:(r + 1) * 128, :],
                                    in_=p_sb)
            nc.gpsimd.collective_compute(
                "AllGather", mybir.AluOpType.bypass,
                replica_groups=[list(range(N_CORES))],
                ins=[p_in.opt()], outs=[p_out.opt()])

            # ---- reload P_0, rebuild T_run = P_0^T ---------------------
            P0 = [p_pool.tile([128, D], bf16, name=f"P0_{t}", tag=f"P{t}")
                  for t in range(NT)]
            for t in range(NT):
                nc.scalar.dma_start(out=P0[t],
                                    in_=p_out[t * 128:(t + 1) * 128, :])
            Trun = [t_pool.tile([128, D], bf16, name=f"Tr0_{t}", tag=f"T{t}")
                    for t in range(NT)]
            for r in range(NT):
                ps = psX.tile([128, D], bf16, name=f"psR_{r}", tag="psx")
                for cb in range(NT):
                    nc.tensor.transpose(
                        ps[:, cb * 128:(cb + 1) * 128],
                        P0[cb][:, r * 128:(r + 1) * 128], ident_sb)
                eng = nc.vector if r % 2 == 0 else nc.scalar
                if eng is nc.vector:
                    eng.tensor_copy(Trun[r], ps)
                else:
                    eng.copy(out=Trun[r], in_=ps)
            # stage all gathered segment c's into SBUF (fp32)
            cs_tiles = []
            for s in range(N_CORES):
                cs = misc.tile([128, NT], f32, name=f"cs_{s}")
                nc.scalar.dma_start(out=cs,
                                    in_=c_out[s * 128:(s + 1) * 128, :])
                cs_tiles.append(cs)
            # c_run starts as segment 0's c (cast to bf16 for the matmuls)
            c_run = c_pool.tile([128, NT], bf16, name="c_r0", tag="c")
            nc.vector.tensor_copy(c_run, cs_tiles[0])
            for s in range(1, N_CORES):
                Ps = [p_pool.tile([128, D], bf16, name=f"Ps{s}_{t}",
                                  tag=f"P{t}") for t in range(NT)]
                for t in range(NT):
                    nc.scalar.dma_start(
                        out=Ps[t],
                        in_=p_out[(s * NT + t) * 128:(s * NT + t + 1) * 128, :])
                Trun, c_run = compose_step(
                    Ps, Trun, c_run, None, cs_tiles[s],
                    last_c_f32=(s == N_CORES - 1), tagsuf=f"cb_{s}")
            c_fin = c_run  # fp32 [128, NT]

            # ---- final transpose: M = T_run^T ([in,out], d-major) ------
            Msb = []
            for r in range(NT):
                ps = psX.tile([128, D], bf16, name=f"psM_{r}", tag="psx")
                for cb in range(NT):
                    nc.tensor.transpose(
                        ps[:, cb * 128:(cb + 1) * 128],
                        Trun[cb][:, r * 128:(r + 1) * 128], ident_sb)
                m_sb = misc.tile([128, D], bf16, name=f"M_{r}")
                eng = nc.vector if r % 2 == 0 else nc.scalar
                if eng is nc.vector:
                    eng.tensor_copy(m_sb, ps)
                else:
                    eng.copy(out=m_sb, in_=ps)
                Msb.append(m_sb)

            # ---- apply: yT[j, b] = sum_d M[d, j] xT[d, b] + c[j] -------
            for bc in range(NB):
                for j in range(NT):
                    ps = psT.tile([128, 512], f32, name=f"psA_{bc}_{j}",
                                  tag=f"psT{j}")
                    for k in range(NT):
                        nc.tensor.matmul(
                            ps, Msb[k][:, j * 128:(j + 1) * 128],
                            Xt[k][:, bc * 512:(bc + 1) * 512],
                            start=(k == 0), stop=(k == NT - 1))
                    y_sb = y_pool.tile([128, 512], f32, name=f"y_{bc}_{j}",
                                       tag=f"y{j}")
                    bias_ap = c_fin[:, j:j + 1]
                    if (bc * NT + j) % 2 == 0:
                        nc.vector.tensor_scalar_add(out=y_sb, in0=ps,
                                                    scalar1=bias_ap)
                    else:
                        nc.scalar.add(out=y_sb, in_=ps, add=bias_ap)
                    nc.sync.dma_start(
                        out=yT[j, :, bc * 512:(bc + 1) * 512], in_=y_sb)

    nc.compile()
    return nc


def _get_nc():
    key = "default"
    if key not in _NC_CACHE:
        _NC_CACHE[key] = _build_nc()
    return _NC_CACHE[key]


def kernel(x: np.ndarray, Ws: np.ndarray, bs: np.ndarray) -> np.ndarray:
    global LAST_EXEC_TIME_NS, LAST_RESULTS
    x = np.ascontiguousarray(np.asarray(x, dtype=np.float32))
    Ws = np.ascontiguousarray(np.asarray(Ws, dtype=np.float32))
    bs = np.ascontiguousarray(np.asarray(bs, dtype=np.float32))

    ident = np.eye(128, dtype=bfloat16)
    in_maps = []
    for i in range(N_CORES):
        l0, l1 = SEG_BOUNDS[i], SEG_BOUNDS[i + 1]
        nlay = l1 - l0
        # T0 = Ws[l0] ([out,in] layout), tiled by rows of 128
        T0 = Ws[l0].astype(bfloat16).reshape(NT, 128, D)
        # compose layers l0+1..l1-1 as d-major W^T, identity padded to NCOMP
        WTs = np.zeros((NCOMP, NT, 128, D), dtype=bfloat16)
        bsT = np.zeros((128, (NCOMP + 1) * NT), dtype=np.float32)
        bsT[:, 0:NT] = bs[l0].reshape(NT, 128).T
        for m in range(NCOMP):
            l = l0 + 1 + m
            if l < l1:
                WTs[m] = Ws[l].T.astype(bfloat16).reshape(NT, 128, D)
                bsT[:, (m + 1) * NT:(m + 2) * NT] = bs[l].reshape(NT, 128).T
            else:
                WTs[m] = np.eye(D, dtype=bfloat16).reshape(NT, 128, D)
        c0 = bs[l0].reshape(NT, 128).T.astype(bfloat16)  # [128, NT]
        shard = x[i * B:(i + 1) * B, :].T  # [D, B]
        xTt = shard.astype(bfloat16).reshape(NT, 128, B)
        in_maps.append({
            "T0": np.ascontiguousarray(T0),
            "WTs": np.ascontiguousarray(WTs),
            "bsT": np.ascontiguousarray(bsT),
            "c0": np.ascontiguousarray(c0),
            "xT": np.ascontiguousarray(xTt),
            "ident": ident,
        })

    nc = _get_nc()
    trace = os.environ.get("BASS_KERNEL_TRACE", "0") == "1"
    res = run_bass_kernel_spmd(nc, in_maps, list(range(N_CORES)), trace=trace)
    LAST_EXEC_TIME_NS = res.exec_time_ns
    LAST_RESULTS = res

    # yT [NT, 128, B] fp32 -> y_shard [B, D]
    shards = []
    for i in range(N_CORES):
        yt = res.results[i]["yT"].reshape(D, B)
        shards.append(yt.T)
    y = np.concatenate(shards, axis=0)
    return np.ascontiguousarray(y.astype(np.float32))
